# revision 6
# baseline (speedup 1.0000x reference)
"""Causal self-attention (dense transformer block) on 8 trn2 NeuronCores.

Sharding: tensor-parallel over heads. Each core owns 2 of the 16 heads:
  - qkv projection: column-slice of W_qkv (128 cols per core)
  - attention for its (2 heads x 2 batches) = 4 (b,h) pairs
  - out projection: row-slice of W_out -> partial y [4096, 1024]
Host sums the 8 partial y's and adds b_out (+ the v-bias term folded
through W_out, since softmax rows sum to 1).

Device pipeline (bf16 matmuls, fp32 accumulation):
  xT [d, s] (host-transposed)  --W stationary-->  qT,kT [hd, s]; vT -> PE
  transpose -> v natural [s, hd] (+ ones col -> v_aug).
  scores sT[k, q] = kT_tile.T @ qT  (K=64), exp on ScalarE (no max
  subtraction: scores are bounded), causal mask via gpsimd affine_select
  on diagonal tiles, PV: attnT[65, q] += v_aug.T @ PT (row 64 = softmax
  denominators), normalize via reciprocal_approx_fast + K=1 broadcast
  matmul, out-proj: y[q, e] = attnT_tile.T @ W_out_rows.
"""

import sys

if "/opt/trn_rl_repo" not in sys.path:
    sys.path.insert(0, "/opt/trn_rl_repo")

import numpy as np
import ml_dtypes

import concourse.bass as bass
import concourse.tile as tile
from concourse import bacc, mybir
from concourse.bass_utils import run_bass_kernel_spmd
from concourse.masks import make_identity

BF16 = mybir.dt.bfloat16
F32 = mybir.dt.float32
AF = mybir.ActivationFunctionType

N_EMBED = 1024
N_HEAD = 16
HEAD_DIM = 64
N_CORES = 8
HEADS_PER_CORE = N_HEAD // N_CORES          # 2
DCORE = HEADS_PER_CORE * HEAD_DIM           # 128 head-dims per core
B = 2
S = 2048                                    # seq len per batch
QB = 512                                    # q-block (moving free dim)
KT = 128                                    # k-tile (contraction tile)
DT = N_EMBED // 128                         # 8 d-tiles for projections
SCALE = 1.0 / 8.0                           # 1/sqrt(HEAD_DIM)


def build_program(seq=S):
    """Build the per-core Bass program (identical on all cores; SPMD)."""
    s_tot = B * seq                 # total rows across batches
    n_qb = seq // QB                # q-blocks per batch
    n_kt = seq // KT                # k-tiles per batch
    n_sb = s_tot // QB              # s-blocks for projections
    kt_per_qb = QB // KT            # 4

    nc = bacc.Bacc("TRN2", target_bir_lowering=False, debug=False,
                   num_devices=N_CORES)

    xT = nc.dram_tensor("xT", [N_EMBED, s_tot], BF16, kind="ExternalInput")
    wq = nc.dram_tensor("wq", [N_EMBED, DCORE], BF16, kind="ExternalInput")
    wk = nc.dram_tensor("wk", [N_EMBED, DCORE], BF16, kind="ExternalInput")
    wv = nc.dram_tensor("wv", [N_EMBED, DCORE], BF16, kind="ExternalInput")
    bq = nc.dram_tensor("bq", [DCORE, 1], F32, kind="ExternalInput")
    bk = nc.dram_tensor("bk", [DCORE, 1], F32, kind="ExternalInput")
    wout = nc.dram_tensor("wout", [DCORE, N_EMBED], BF16, kind="ExternalInput")
    y = nc.dram_tensor("y", [s_tot, N_EMBED], BF16, kind="ExternalOutput")

    with tile.TileContext(nc) as tc:
        with (
            tc.tile_pool(name="singles", bufs=1) as singles,
            tc.tile_pool(name="proj_ps", bufs=3, space="PSUM") as proj_pool,
            tc.tile_pool(name="tr_ps", bufs=2, space="PSUM") as tr_pool,
            tc.tile_pool(name="vstage", bufs=2) as vstage_pool,
        ):
            # ---- persistent SBUF tensors ----
            xT_sb = singles.tile([128, DT, s_tot], BF16)
            wq_sb = singles.tile([128, DT, DCORE], BF16)
            wk_sb = singles.tile([128, DT, DCORE], BF16)
            wv_sb = singles.tile([128, DT, DCORE], BF16)
            bq_sb = singles.tile([DCORE, 1], F32)
            bk_sb = singles.tile([DCORE, 1], F32)
            wout_sb = singles.tile([DCORE, N_EMBED], BF16)
            qT_sb = singles.tile([DCORE, s_tot], BF16)
            kT_sb = singles.tile([DCORE, s_tot], BF16)
            # v_aug: per global k-tile kt: [v_h0 | 1 | v_h1 | 1] (65+65 cols)
            v_aug = singles.tile([128, B * n_kt, 2 * (HEAD_DIM + 1)], BF16)
            attnT_sb = singles.tile([DCORE, B, seq], BF16)
            ident_sb = singles.tile([128, 128], BF16)

            # ---- input DMAs ----
            nc.sync.dma_start(out=xT_sb,
                              in_=xT.ap().rearrange("(t p) s -> p t s", p=128))
            nc.sync.dma_start(out=wq_sb,
                              in_=wq.ap().rearrange("(t p) h -> p t h", p=128))
            nc.sync.dma_start(out=wk_sb,
                              in_=wk.ap().rearrange("(t p) h -> p t h", p=128))
            nc.sync.dma_start(out=wv_sb,
                              in_=wv.ap().rearrange("(t p) h -> p t h", p=128))
            nc.sync.dma_start(out=bq_sb, in_=bq.ap())
            nc.sync.dma_start(out=bk_sb, in_=bk.ap())
            nc.sync.dma_start(out=wout_sb, in_=wout.ap())

            make_identity(nc, ident_sb)
            nc.vector.memset(v_aug[:, :, HEAD_DIM], 1.0)
            nc.vector.memset(v_aug[:, :, 2 * HEAD_DIM + 1], 1.0)

            # ---- phase 1: projections ----
            for j in range(n_sb):
                sl = slice(j * QB, (j + 1) * QB)
                for w_sb, b_sb, dst in ((wq_sb, bq_sb, qT_sb),
                                        (wk_sb, bk_sb, kT_sb)):
                    ps = proj_pool.tile([128, QB], F32, tag="proj")
                    for t in range(DT):
                        nc.tensor.matmul(ps, lhsT=w_sb[:, t, :],
                                         rhs=xT_sb[:, t, sl],
                                         start=(t == 0), stop=(t == DT - 1))
                    nc.scalar.activation(dst[:, sl], ps, AF.Identity,
                                         bias=b_sb, scale=1.0)
                ps = proj_pool.tile([128, QB], F32, tag="proj")
                for t in range(DT):
                    nc.tensor.matmul(ps, lhsT=wv_sb[:, t, :],
                                     rhs=xT_sb[:, t, sl],
                                     start=(t == 0), stop=(t == DT - 1))
                vstage = vstage_pool.tile([128, QB], BF16)
                nc.vector.tensor_copy(vstage, ps)
                for u in range(QB // 128):
                    kt_gl = (QB // 128) * j + u
                    tr = tr_pool.tile([128, 128], BF16)
                    nc.tensor.transpose(tr, vstage[:, u * 128:(u + 1) * 128],
                                        ident_sb)
                    nc.vector.tensor_copy(v_aug[:, kt_gl, 0:HEAD_DIM],
                                          tr[:, 0:HEAD_DIM])
                    nc.vector.tensor_copy(
                        v_aug[:, kt_gl, HEAD_DIM + 1:2 * HEAD_DIM + 1],
                        tr[:, HEAD_DIM:2 * HEAD_DIM])

        # ---- phase 2: attention ----
        with (
            tc.tile_pool(name="score_ps", bufs=2, space="PSUM") as score_pool,
            tc.tile_pool(name="attn_ps", bufs=2, space="PSUM") as attn_pool,
            tc.tile_pool(name="bc_sb", bufs=2) as bc_pool,
            tc.tile_pool(name="pt_sb", bufs=3) as pt_pool,
            tc.tile_pool(name="rec_sb", bufs=2) as rec_pool,
        ):
            for b_i in range(B):
                for h in range(HEADS_PER_CORE):
                    hsl = slice(HEAD_DIM * h, HEAD_DIM * (h + 1))
                    for j in range(n_qb):
                        qsl = slice(b_i * seq + j * QB, b_i * seq + (j + 1) * QB)
                        attn_ps = attn_pool.tile([HEAD_DIM + 1, QB], F32)
                        nkt = kt_per_qb * (j + 1)
                        for g in range(nkt // 2):
                            s_ps = score_pool.tile([128, 2 * QB], F32)
                            pt = pt_pool.tile([128, 2 * QB], BF16)
                            for u in range(2):
                                kt = 2 * g + u
                                ks = slice(b_i * seq + kt * 128,
                                           b_i * seq + kt * 128 + 128)
                                nc.tensor.matmul(
                                    s_ps[:, u * QB:(u + 1) * QB],
                                    lhsT=kT_sb[hsl, ks], rhs=qT_sb[hsl, qsl],
                                    start=True, stop=True)
                            nc.scalar.activation(pt, s_ps, AF.Exp, scale=SCALE)
                            for u in range(2):
                                kt = 2 * g + u
                                if kt >= kt_per_qb * j:  # diagonal tile
                                    nc.gpsimd.affine_select(
                                        out=pt[:, u * QB:(u + 1) * QB],
                                        in_=pt[:, u * QB:(u + 1) * QB],
                                        compare_op=mybir.AluOpType.is_ge,
                                        fill=0.0,
                                        base=QB * j - 128 * kt,
                                        channel_multiplier=-1,
                                        pattern=[[1, QB]])
                            for u in range(2):
                                kt = 2 * g + u
                                nc.tensor.matmul(
                                    attn_ps,
                                    lhsT=v_aug[:, b_i * n_kt + kt,
                                               (HEAD_DIM + 1) * h:
                                               (HEAD_DIM + 1) * (h + 1)],
                                    rhs=pt[:, u * QB:(u + 1) * QB],
                                    start=(kt == 0), stop=(kt == nkt - 1))
                        # normalize: recip of sums row, broadcast, multiply
                        # sums row (psum p64) -> sbuf p0, recip, broadcast
                        r0 = rec_pool.tile([1, QB], F32)
                        nc.vector.tensor_copy(
                            r0, attn_ps[HEAD_DIM:HEAD_DIM + 1, :])
                        rf = rec_pool.tile([1, QB], F32, tag="rf")
                        nc.vector.reciprocal_approx_fast(rf, r0)
                        bc_sb = bc_pool.tile([HEAD_DIM, QB], F32)
                        nc.gpsimd.partition_broadcast(bc_sb, rf)
                        nc.vector.tensor_mul(
                            attnT_sb[hsl, b_i, j * QB:(j + 1) * QB],
                            attn_ps[0:HEAD_DIM, :], bc_sb)

        # ---- phase 3: out projection ----
        with (
            tc.tile_pool(name="y_ps", bufs=2, space="PSUM") as y_pool,
            tc.tile_pool(name="y_sb", bufs=3) as ysb_pool,
        ):
            for b_i in range(B):
                for qt in range(seq // 128):
                    at = attnT_sb[:, b_i, qt * 128:(qt + 1) * 128]
                    y_ps = y_pool.tile([128, N_EMBED], F32)
                    for u in range(N_EMBED // QB):
                        nc.tensor.matmul(y_ps[:, u * QB:(u + 1) * QB],
                                         lhsT=at, rhs=wout_sb[:, u * QB:(u + 1) * QB],
                                         start=True, stop=True)
                    y_sb = ysb_pool.tile([128, N_EMBED], BF16)
                    if qt % 2 == 0:
                        nc.vector.tensor_copy(y_sb, y_ps)
                    else:
                        nc.scalar.copy(y_sb, y_ps)
                    nc.sync.dma_start(
                        out=y.ap()[b_i * seq + qt * 128:
                                   b_i * seq + (qt + 1) * 128, :],
                        in_=y_sb)

    nc.compile()
    return nc


_CACHE = {}


def _get_program(seq=S):
    if seq not in _CACHE:
        _CACHE[seq] = build_program(seq)
    return _CACHE[seq]


def make_in_maps(x, W_qkv, b_qkv, seq=S):
    bf16 = ml_dtypes.bfloat16
    s_tot = B * seq
    xT = np.ascontiguousarray(
        x.reshape(s_tot, N_EMBED).T).astype(bf16)
    in_maps = []
    for c in range(N_CORES):
        csl = slice(DCORE * c, DCORE * (c + 1))
        in_maps.append({
            "xT": xT,
            "wq": np.ascontiguousarray(W_qkv[:, csl]).astype(bf16),
            "wk": np.ascontiguousarray(W_qkv[:, N_EMBED:][:, csl]).astype(bf16),
            "wv": np.ascontiguousarray(W_qkv[:, 2 * N_EMBED:][:, csl]).astype(bf16),
            "bq": np.ascontiguousarray(
                b_qkv[csl].reshape(DCORE, 1)).astype(np.float32),
            "bk": np.ascontiguousarray(
                b_qkv[N_EMBED:][csl].reshape(DCORE, 1)).astype(np.float32),
            "wout": None,  # filled below
        })
    return in_maps


def kernel(x, W_qkv, b_qkv, W_out, b_out):
    x = np.asarray(x, dtype=np.float32)
    W_qkv = np.asarray(W_qkv, dtype=np.float32)
    b_qkv = np.asarray(b_qkv, dtype=np.float32)
    W_out = np.asarray(W_out, dtype=np.float32)
    b_out = np.asarray(b_out, dtype=np.float32)

    nc = _get_program(S)
    in_maps = make_in_maps(x, W_qkv, b_qkv, S)
    bf16 = ml_dtypes.bfloat16
    for c in range(N_CORES):
        csl = slice(DCORE * c, DCORE * (c + 1))
        in_maps[c]["wout"] = np.ascontiguousarray(W_out[csl, :]).astype(bf16)

    res = run_bass_kernel_spmd(nc, in_maps, core_ids=list(range(N_CORES)))
    y = np.zeros((B * S, N_EMBED), dtype=np.float32)
    for r in res.results:
        y += r["y"].astype(np.float32)
    # bias + v-bias folded through W_out (softmax rows sum to 1)
    y += b_out[None, :] + b_qkv[2 * N_EMBED:] @ W_out
    return y.reshape(B, S, N_EMBED)


# revision 18
# speedup vs baseline: 1.3602x; 1.3602x over previous
"""Causal self-attention (dense transformer block) on 8 trn2 NeuronCores.

Sharding: tensor-parallel over heads. Each core owns 2 of the 16 heads:
  - qkv projection: column-slice of W_qkv (128 cols per core)
  - attention for its (2 heads x 2 batches) = 4 (b,h) pairs
  - out projection: row-slice of W_out -> partial y [4096, 1024]
Host sums the 8 partial y's and adds b_out (+ the v-bias term folded
through W_out, since softmax rows sum to 1).

Device pipeline (bf16 matmuls, fp32 accumulation), interleaved per
(batch, 512-row block) so PE/ACT/DVE/GPSIMD overlap:
  proj block:  qT,kT[hd, s] = W.T @ xT (+bias, DVE); v via PE-transpose
               of vT -> v_aug [v_h0 | 1 | v_h1 | 1]
  attn block:  per k-tile: sT[k, q] both heads side by side in one PSUM
               tile, one exp (ScalarE, no max subtraction: scores are
               bounded), causal mask via one gpsimd affine_select on
               diagonal tiles, PV: attnT[65, q] += v_aug.T @ PT
               (row 64 = softmax denominators), normalize via
               reciprocal_approx_fast + partition_broadcast,
  out-proj:    y[q, e] = attnT_tile.T @ W_out_rows, evict, store.
"""

import sys

if "/opt/trn_rl_repo" not in sys.path:
    sys.path.insert(0, "/opt/trn_rl_repo")

import numpy as np
import ml_dtypes

import concourse.bass as bass
import concourse.tile as tile
from concourse import bacc, mybir
from concourse.bass_utils import run_bass_kernel_spmd
from concourse.masks import make_identity

BF16 = mybir.dt.bfloat16
F32 = mybir.dt.float32
AF = mybir.ActivationFunctionType

N_EMBED = 1024
N_HEAD = 16
HEAD_DIM = 64
N_CORES = 8
HEADS_PER_CORE = N_HEAD // N_CORES          # 2
DCORE = HEADS_PER_CORE * HEAD_DIM           # 128 head-dims per core
B = 2
S = 2048                                    # seq len per batch
QB = 512                                    # q-block (moving free dim)
KT = 128                                    # k-tile (contraction tile)
DT = N_EMBED // 128                         # 8 d-tiles for projections
SCALE = 1.0 / 8.0                           # 1/sqrt(HEAD_DIM)
VW = HEAD_DIM + 1                           # v_aug slice width per head
H = HEADS_PER_CORE


def build_program(seq=S):
    """Build the per-core Bass program (identical on all cores; SPMD)."""
    s_tot = B * seq                 # total rows across batches
    n_qb = seq // QB                # q-blocks per batch
    n_kt = seq // KT                # k-tiles per batch
    kt_per_qb = QB // KT            # 4

    nc = bacc.Bacc("TRN2", target_bir_lowering=False, debug=False,
                   num_devices=N_CORES)

    xT = nc.dram_tensor("xT", [N_EMBED, s_tot], BF16, kind="ExternalInput")
    wq = nc.dram_tensor("wq", [N_EMBED, DCORE], BF16, kind="ExternalInput")
    wk = nc.dram_tensor("wk", [N_EMBED, DCORE], BF16, kind="ExternalInput")
    wv = nc.dram_tensor("wv", [N_EMBED, DCORE], BF16, kind="ExternalInput")
    bq = nc.dram_tensor("bq", [DCORE, 1], F32, kind="ExternalInput")
    bk = nc.dram_tensor("bk", [DCORE, 1], F32, kind="ExternalInput")
    wout = nc.dram_tensor("wout", [DCORE, N_EMBED], BF16, kind="ExternalInput")
    y = nc.dram_tensor("y", [s_tot, N_EMBED], BF16, kind="ExternalOutput")

    xT_r = xT.ap().rearrange("(t p) s -> p t s", p=128)

    with (
        tile.TileContext(nc) as tc,
        tc.tile_pool(name="singles", bufs=1) as singles,
        # PSUM (8 banks): sy 2x[128,1024]=4, attn 2x[65,512]=2, pv 2x1=2
        tc.tile_pool(name="sy_ps", bufs=2, space="PSUM") as sy_pool,
        tc.tile_pool(name="attn_ps", bufs=1, space="PSUM") as attn_pool,
        tc.tile_pool(name="pv_ps", bufs=2, space="PSUM") as pv_pool,
        tc.tile_pool(name="vstage", bufs=2) as vstage_pool,
        tc.tile_pool(name="pt_sb", bufs=4) as pt_pool,
        tc.tile_pool(name="rec_sb", bufs=2) as rec_pool,
        tc.tile_pool(name="bc_sb", bufs=2) as bc_pool,
        tc.tile_pool(name="at_sb", bufs=3) as at_pool,
        tc.tile_pool(name="y_sb", bufs=4) as ysb_pool,
    ):
        # ---- persistent SBUF tensors ----
        xT_sb = singles.tile([128, DT, s_tot], BF16)
        wq_sb = singles.tile([128, DT, DCORE], BF16)
        wk_sb = singles.tile([128, DT, DCORE], BF16)
        wv_sb = singles.tile([128, DT, DCORE], BF16)
        bq_sb = singles.tile([DCORE, 1], F32)
        bk_sb = singles.tile([DCORE, 1], F32)
        wout_sb = singles.tile([DCORE, N_EMBED], BF16)
        qT_sb = singles.tile([DCORE, s_tot], BF16)
        kT_sb = singles.tile([DCORE, s_tot], BF16)
        # v_aug: per global k-tile kt: [v_h0 | 1 | v_h1 | 1]
        v_aug = singles.tile([128, B * n_kt, 2 * VW], BF16)
        ident_sb = singles.tile([128, 128], BF16)

        # ---- input DMAs (xT split per d-tile so compute can start) ----
        nc.sync.dma_start(out=wq_sb,
                          in_=wq.ap().rearrange("(t p) h -> p t h", p=128))
        nc.sync.dma_start(out=wk_sb,
                          in_=wk.ap().rearrange("(t p) h -> p t h", p=128))
        nc.sync.dma_start(out=wv_sb,
                          in_=wv.ap().rearrange("(t p) h -> p t h", p=128))
        nc.sync.dma_start(out=bq_sb, in_=bq.ap())
        nc.sync.dma_start(out=bk_sb, in_=bk.ap())
        nc.sync.dma_start(out=wout_sb, in_=wout.ap())
        # column-wise: s-block sb's full-depth slice arrives together,
        # so proj/attention of block 0 start ~1MB into the load
        for sb in range(s_tot // QB):
            nc.sync.dma_start(out=xT_sb[:, :, sb * QB:(sb + 1) * QB],
                              in_=xT_r[:, :, sb * QB:(sb + 1) * QB])

        make_identity(nc, ident_sb)
        nc.vector.memset(v_aug[:, :, HEAD_DIM], 1.0)
        nc.vector.memset(v_aug[:, :, 2 * HEAD_DIM + 1], 1.0)

        def proj_block(sb):
            """Projections for 512-row block sb (global)."""
            sl = slice(sb * QB, (sb + 1) * QB)
            for w_sb, b_sb, dst in ((wq_sb, bq_sb, qT_sb),
                                    (wk_sb, bk_sb, kT_sb)):
                ps = pv_pool.tile([128, QB], F32, tag="aux", name="proj")
                for t in range(DT):
                    nc.tensor.matmul(ps, lhsT=w_sb[:, t, :],
                                     rhs=xT_sb[:, t, sl],
                                     start=(t == 0), stop=(t == DT - 1))
                nc.vector.tensor_scalar_add(dst[:, sl], ps, b_sb)
            ps = pv_pool.tile([128, QB], F32, tag="aux", name="proj")
            for t in range(DT):
                nc.tensor.matmul(ps, lhsT=wv_sb[:, t, :],
                                 rhs=xT_sb[:, t, sl],
                                 start=(t == 0), stop=(t == DT - 1))
            vstage = vstage_pool.tile([128, QB], BF16)
            nc.vector.tensor_copy(vstage, ps)
            for u in range(QB // 128):
                kt_gl = (QB // 128) * sb + u
                tr = pv_pool.tile([128, 128], BF16, tag="aux", name="tr")
                nc.tensor.transpose(tr, vstage[:, u * 128:(u + 1) * 128],
                                    ident_sb)
                nc.vector.tensor_copy(v_aug[:, kt_gl, 0:HEAD_DIM],
                                      tr[:, 0:HEAD_DIM])
                nc.vector.tensor_copy(
                    v_aug[:, kt_gl, HEAD_DIM + 1:2 * HEAD_DIM + 1],
                    tr[:, HEAD_DIM:2 * HEAD_DIM])

        def attn_kloop(b_i, j):
            """Score/exp/mask/PV loop for q-block j of batch b_i.

            Diagonal k-tiles first so the gpsimd masks run while the
            off-diagonal matmuls proceed. Returns evicted (at64, r0)
            SBUF tiles per head for the deferred normalization."""
            qsl = slice(b_i * seq + j * QB, b_i * seq + (j + 1) * QB)
            attn_ps = [attn_pool.tile([VW, QB], F32, tag=f"attn{h}",
                                      name=f"attn{h}") for h in range(H)]
            kts = list(range(kt_per_qb * j, kt_per_qb * (j + 1))) + \
                list(range(0, kt_per_qb * j))
            for pos, kt in enumerate(kts):
                ks = slice(b_i * seq + kt * 128, b_i * seq + kt * 128 + 128)
                d = kt - kt_per_qb * j
                off = 128 * d if d >= 0 else 0   # first valid q column
                s_ps = sy_pool.tile([128, H, QB], F32, tag="sy", name="s_ps")
                pt = pt_pool.tile([128, H, QB], BF16, tag="pt", name="pt")
                for h in range(H):
                    hsl = slice(HEAD_DIM * h, HEAD_DIM * (h + 1))
                    nc.tensor.matmul(
                        s_ps[:, h, off:],
                        lhsT=kT_sb[hsl, ks],
                        rhs=qT_sb[hsl, qsl.start + off:qsl.stop],
                        start=True, stop=True)
                nc.scalar.activation(pt[:, :, off:], s_ps[:, :, off:],
                                     AF.Exp, scale=SCALE)
                if d >= 0:  # diagonal: mask both heads at once
                    nc.gpsimd.affine_select(
                        out=pt[:, :, off:], in_=pt[:, :, off:],
                        compare_op=mybir.AluOpType.is_ge, fill=0.0,
                        base=0, channel_multiplier=-1,
                        pattern=[[0, H], [1, QB - off]])
                for h in range(H):
                    nc.tensor.matmul(
                        attn_ps[h][:, off:],
                        lhsT=v_aug[:, b_i * n_kt + kt, VW * h:VW * (h + 1)],
                        rhs=pt[:, h, off:],
                        start=(pos == 0), stop=(pos == len(kts) - 1))
            # evict accumulators to SBUF to free the PSUM banks
            evicted = []
            for h in range(H):
                at64 = at_pool.tile([HEAD_DIM, QB], F32, tag=f"at64{h}",
                                    name=f"at64{h}")
                nc.vector.tensor_copy(at64, attn_ps[h][0:HEAD_DIM, :])
                r0 = rec_pool.tile([1, QB], F32, tag=f"r0{h}", name=f"r0{h}")
                nc.vector.tensor_copy(r0, attn_ps[h][HEAD_DIM:HEAD_DIM + 1, :])
                evicted.append((at64, r0))
            return evicted

        def norm_outproj(b_i, j, evicted):
            """Deferred normalization + out-projection for q-block j."""
            at_bj = at_pool.tile([DCORE, QB], BF16, name="at_bj")
            for h, (at64, r0) in enumerate(evicted):
                rf = rec_pool.tile([1, QB], F32, tag=f"rf{h}", name=f"rf{h}")
                nc.vector.reciprocal_approx_fast(rf, r0)
                bc_sb = bc_pool.tile([HEAD_DIM, QB], F32, tag=f"bc{h}",
                                     name=f"bc{h}")
                nc.gpsimd.partition_broadcast(bc_sb, rf)
                nc.vector.tensor_mul(
                    at_bj[HEAD_DIM * h:HEAD_DIM * (h + 1), :], at64, bc_sb)
            for qt in range(QB // 128):
                at = at_bj[:, qt * 128:(qt + 1) * 128]
                ysb = ysb_pool.tile([128, N_EMBED], BF16, tag="ysb",
                                    name="ysb")
                for u in range(N_EMBED // QB):
                    yp = pv_pool.tile([128, QB], F32, tag="aux", name="yp")
                    nc.tensor.matmul(yp, lhsT=at,
                                     rhs=wout_sb[:, u * QB:(u + 1) * QB],
                                     start=True, stop=True)
                    if u == 0:
                        nc.vector.tensor_copy(ysb[:, 0:QB], yp)
                    else:
                        nc.scalar.copy(ysb[:, QB:2 * QB], yp)
                row0 = b_i * seq + j * QB + qt * 128
                nc.sync.dma_start(out=y.ap()[row0:row0 + 128, :], in_=ysb)

        # ---- interleaved schedule: proj frontloaded 3 blocks ahead,
        # norm/out-proj one block behind ----
        n_blocks = B * n_qb
        next_proj = 0
        for _ in range(3):
            if next_proj < n_blocks:
                proj_block(next_proj)
                next_proj += 1
        pending = None
        for b_i in range(B):
            for j in range(n_qb):
                if next_proj < n_blocks:
                    proj_block(next_proj)
                    next_proj += 1
                evicted = attn_kloop(b_i, j)
                if pending is not None:
                    norm_outproj(*pending)
                pending = (b_i, j, evicted)
        norm_outproj(*pending)

    nc.compile()
    return nc


_CACHE = {}


def _get_program(seq=S):
    if seq not in _CACHE:
        _CACHE[seq] = build_program(seq)
    return _CACHE[seq]


def make_in_maps(x, W_qkv, b_qkv, seq=S):
    bf16 = ml_dtypes.bfloat16
    s_tot = B * seq
    xT = np.ascontiguousarray(
        x.reshape(s_tot, N_EMBED).T).astype(bf16)
    in_maps = []
    for c in range(N_CORES):
        csl = slice(DCORE * c, DCORE * (c + 1))
        in_maps.append({
            "xT": xT,
            "wq": np.ascontiguousarray(W_qkv[:, csl]).astype(bf16),
            "wk": np.ascontiguousarray(W_qkv[:, N_EMBED:][:, csl]).astype(bf16),
            "wv": np.ascontiguousarray(W_qkv[:, 2 * N_EMBED:][:, csl]).astype(bf16),
            "bq": np.ascontiguousarray(
                b_qkv[csl].reshape(DCORE, 1)).astype(np.float32),
            "bk": np.ascontiguousarray(
                b_qkv[N_EMBED:][csl].reshape(DCORE, 1)).astype(np.float32),
            "wout": None,  # filled by caller
        })
    return in_maps


def kernel(x, W_qkv, b_qkv, W_out, b_out):
    x = np.asarray(x, dtype=np.float32)
    W_qkv = np.asarray(W_qkv, dtype=np.float32)
    b_qkv = np.asarray(b_qkv, dtype=np.float32)
    W_out = np.asarray(W_out, dtype=np.float32)
    b_out = np.asarray(b_out, dtype=np.float32)

    nc = _get_program(S)
    in_maps = make_in_maps(x, W_qkv, b_qkv, S)
    bf16 = ml_dtypes.bfloat16
    for c in range(N_CORES):
        csl = slice(DCORE * c, DCORE * (c + 1))
        in_maps[c]["wout"] = np.ascontiguousarray(W_out[csl, :]).astype(bf16)

    res = run_bass_kernel_spmd(nc, in_maps, core_ids=list(range(N_CORES)))
    y = np.zeros((B * S, N_EMBED), dtype=np.float32)
    for r in res.results:
        y += r["y"].astype(np.float32)
    # bias + v-bias folded through W_out (softmax rows sum to 1)
    y += b_out[None, :] + b_qkv[2 * N_EMBED:] @ W_out
    return y.reshape(B, S, N_EMBED)


# revision 21
# speedup vs baseline: 1.3792x; 1.0140x over previous
"""Causal self-attention (dense transformer block) on 8 trn2 NeuronCores.

Sharding: tensor-parallel over heads. Each core owns 2 of the 16 heads:
  - qkv projection: column-slice of W_qkv (128 cols per core)
  - attention for its (2 heads x 2 batches) = 4 (b,h) pairs
  - out projection: row-slice of W_out -> partial y [4096, 1024]
Host sums the 8 partial y's and adds b_out (+ the v-bias term folded
through W_out, since softmax rows sum to 1).

Device pipeline (bf16 matmuls, fp32 accumulation), interleaved per
(batch, 512-row block) so PE/ACT/DVE/GPSIMD overlap:
  proj block:  qT,kT[hd, s] = W.T @ xT (+bias, DVE); v via PE-transpose
               of vT -> v_aug [v_h0 | 1 | v_h1 | 1]
  attn block:  per k-tile: sT[k, q] both heads side by side in one PSUM
               tile, one exp (ScalarE, no max subtraction: scores are
               bounded), causal mask via one gpsimd affine_select on
               diagonal tiles, PV: attnT[65, q] += v_aug.T @ PT
               (row 64 = softmax denominators), normalize via
               reciprocal_approx_fast + partition_broadcast,
  out-proj:    y[q, e] = attnT_tile.T @ W_out_rows, evict, store.
"""

import sys

if "/opt/trn_rl_repo" not in sys.path:
    sys.path.insert(0, "/opt/trn_rl_repo")

import numpy as np
import ml_dtypes

import concourse.bass as bass
import concourse.tile as tile
from concourse import bacc, mybir
from concourse.bass_utils import run_bass_kernel_spmd
from concourse.masks import make_identity

BF16 = mybir.dt.bfloat16
F32 = mybir.dt.float32
AF = mybir.ActivationFunctionType

N_EMBED = 1024
N_HEAD = 16
HEAD_DIM = 64
N_CORES = 8
HEADS_PER_CORE = N_HEAD // N_CORES          # 2
DCORE = HEADS_PER_CORE * HEAD_DIM           # 128 head-dims per core
B = 2
S = 2048                                    # seq len per batch
QB = 512                                    # q-block (moving free dim)
KT = 128                                    # k-tile (contraction tile)
DT = N_EMBED // 128                         # 8 d-tiles for projections
SCALE = 1.0 / 8.0                           # 1/sqrt(HEAD_DIM)
VW = HEAD_DIM + 1                           # v_aug slice width per head
H = HEADS_PER_CORE


def build_program(seq=S):
    """Build the per-core Bass program (identical on all cores; SPMD)."""
    s_tot = B * seq                 # total rows across batches
    n_qb = seq // QB                # q-blocks per batch
    n_kt = seq // KT                # k-tiles per batch
    kt_per_qb = QB // KT            # 4

    nc = bacc.Bacc("TRN2", target_bir_lowering=False, debug=False,
                   num_devices=N_CORES)

    xT = nc.dram_tensor("xT", [N_EMBED, s_tot], BF16, kind="ExternalInput")
    wq = nc.dram_tensor("wq", [N_EMBED, DCORE], BF16, kind="ExternalInput")
    wk = nc.dram_tensor("wk", [N_EMBED, DCORE], BF16, kind="ExternalInput")
    wv = nc.dram_tensor("wv", [N_EMBED, DCORE], BF16, kind="ExternalInput")
    bq = nc.dram_tensor("bq", [DCORE, 1], F32, kind="ExternalInput")
    bk = nc.dram_tensor("bk", [DCORE, 1], F32, kind="ExternalInput")
    wout = nc.dram_tensor("wout", [DCORE, N_EMBED], BF16, kind="ExternalInput")
    y = nc.dram_tensor("y", [s_tot, N_EMBED], BF16, kind="ExternalOutput")

    xT_r = xT.ap().rearrange("(t p) s -> p t s", p=128)

    with (
        tile.TileContext(nc) as tc,
        tc.tile_pool(name="singles", bufs=1) as singles,
        # PSUM (8 banks): sy 2x[128,1024]=4, attn 2x[65,512]=2, pv 2x1=2
        tc.tile_pool(name="sy_ps", bufs=2, space="PSUM") as sy_pool,
        tc.tile_pool(name="attn_ps", bufs=1, space="PSUM") as attn_pool,
        tc.tile_pool(name="pv_ps", bufs=2, space="PSUM") as pv_pool,
        tc.tile_pool(name="vstage", bufs=2) as vstage_pool,
        tc.tile_pool(name="pt_sb", bufs=6) as pt_pool,
        tc.tile_pool(name="rec_sb", bufs=2) as rec_pool,
        tc.tile_pool(name="bc_sb", bufs=2) as bc_pool,
        tc.tile_pool(name="at_sb", bufs=3) as at_pool,
        tc.tile_pool(name="y_sb", bufs=4) as ysb_pool,
    ):
        # ---- persistent SBUF tensors ----
        xT_sb = singles.tile([128, DT, s_tot], BF16)
        wq_sb = singles.tile([128, DT, DCORE], BF16)
        wk_sb = singles.tile([128, DT, DCORE], BF16)
        wv_sb = singles.tile([128, DT, DCORE], BF16)
        bq_sb = singles.tile([DCORE, 1], F32)
        bk_sb = singles.tile([DCORE, 1], F32)
        wout_sb = singles.tile([DCORE, N_EMBED], BF16)
        qT_sb = singles.tile([DCORE, s_tot], BF16)
        kT_sb = singles.tile([DCORE, s_tot], BF16)
        # v_aug: per global k-tile kt: [v_h0 | 1 | v_h1 | 1]
        v_aug = singles.tile([128, B * n_kt, 2 * VW], BF16)
        ident_sb = singles.tile([128, 128], BF16)

        # ---- input DMAs (xT split per d-tile so compute can start) ----
        nc.sync.dma_start(out=wq_sb,
                          in_=wq.ap().rearrange("(t p) h -> p t h", p=128))
        nc.sync.dma_start(out=wk_sb,
                          in_=wk.ap().rearrange("(t p) h -> p t h", p=128))
        nc.sync.dma_start(out=wv_sb,
                          in_=wv.ap().rearrange("(t p) h -> p t h", p=128))
        nc.sync.dma_start(out=bq_sb, in_=bq.ap())
        nc.sync.dma_start(out=bk_sb, in_=bk.ap())
        nc.sync.dma_start(out=wout_sb, in_=wout.ap())
        # column-wise: s-block sb's full-depth slice arrives together,
        # so proj/attention of block 0 start ~1MB into the load
        for sb in range(s_tot // QB):
            nc.sync.dma_start(out=xT_sb[:, :, sb * QB:(sb + 1) * QB],
                              in_=xT_r[:, :, sb * QB:(sb + 1) * QB])

        make_identity(nc, ident_sb)
        nc.vector.memset(v_aug[:, :, HEAD_DIM], 1.0)
        nc.vector.memset(v_aug[:, :, 2 * HEAD_DIM + 1], 1.0)

        def proj_block(sb):
            """Projections for 512-row block sb (global)."""
            sl = slice(sb * QB, (sb + 1) * QB)
            for w_sb, b_sb, dst in ((wq_sb, bq_sb, qT_sb),
                                    (wk_sb, bk_sb, kT_sb)):
                ps = pv_pool.tile([128, QB], F32, tag="aux", name="proj")
                for t in range(DT):
                    nc.tensor.matmul(ps, lhsT=w_sb[:, t, :],
                                     rhs=xT_sb[:, t, sl],
                                     start=(t == 0), stop=(t == DT - 1))
                nc.vector.tensor_scalar_add(dst[:, sl], ps, b_sb)
            ps = pv_pool.tile([128, QB], F32, tag="aux", name="proj")
            for t in range(DT):
                nc.tensor.matmul(ps, lhsT=wv_sb[:, t, :],
                                 rhs=xT_sb[:, t, sl],
                                 start=(t == 0), stop=(t == DT - 1))
            vstage = vstage_pool.tile([128, QB], BF16)
            nc.vector.tensor_copy(vstage, ps)
            for u in range(QB // 128):
                kt_gl = (QB // 128) * sb + u
                tr = pv_pool.tile([128, 128], BF16, tag="aux", name="tr")
                nc.tensor.transpose(tr, vstage[:, u * 128:(u + 1) * 128],
                                    ident_sb)
                nc.vector.tensor_copy(v_aug[:, kt_gl, 0:HEAD_DIM],
                                      tr[:, 0:HEAD_DIM])
                nc.vector.tensor_copy(
                    v_aug[:, kt_gl, HEAD_DIM + 1:2 * HEAD_DIM + 1],
                    tr[:, HEAD_DIM:2 * HEAD_DIM])

        def attn_kloop(b_i, j):
            """Score/exp/mask/PV loop for q-block j of batch b_i.

            Diagonal k-tiles first so the gpsimd masks run while the
            off-diagonal matmuls proceed. Returns evicted (at64, r0)
            SBUF tiles per head for the deferred normalization."""
            qsl = slice(b_i * seq + j * QB, b_i * seq + (j + 1) * QB)
            attn_ps = [attn_pool.tile([VW, QB], F32, tag=f"attn{h}",
                                      name=f"attn{h}") for h in range(H)]
            kts = list(range(kt_per_qb * j, kt_per_qb * (j + 1))) + \
                list(range(0, kt_per_qb * j))

            def emit_pv(kt, pt, off, pos):
                for h in range(H):
                    nc.tensor.matmul(
                        attn_ps[h][:, off:],
                        lhsT=v_aug[:, b_i * n_kt + kt, VW * h:VW * (h + 1)],
                        rhs=pt[:, h, off:],
                        start=(pos == 0), stop=(pos == len(kts) - 1))

            prev = None
            for pos, kt in enumerate(kts):
                ks = slice(b_i * seq + kt * 128, b_i * seq + kt * 128 + 128)
                d = kt - kt_per_qb * j
                off = 128 * d if d >= 0 else 0   # first valid q column
                s_ps = sy_pool.tile([128, H, QB], F32, tag="sy", name="s_ps")
                pt = pt_pool.tile([128, H, QB], BF16, tag="pt", name="pt")
                for h in range(H):
                    hsl = slice(HEAD_DIM * h, HEAD_DIM * (h + 1))
                    nc.tensor.matmul(
                        s_ps[:, h, off:],
                        lhsT=kT_sb[hsl, ks],
                        rhs=qT_sb[hsl, qsl.start + off:qsl.stop],
                        start=True, stop=True)
                nc.scalar.activation(pt[:, :, off:], s_ps[:, :, off:],
                                     AF.Exp, scale=SCALE)
                if d >= 0:  # diagonal: mask both heads at once
                    nc.gpsimd.affine_select(
                        out=pt[:, :, off:], in_=pt[:, :, off:],
                        compare_op=mybir.AluOpType.is_ge, fill=0.0,
                        base=0, channel_multiplier=-1,
                        pattern=[[0, H], [1, QB - off]])
                # PV deferred one k-tile: scores(kt+1) runs on PE while
                # exp(kt) is still on ScalarE
                if prev is not None:
                    emit_pv(*prev)
                prev = (kt, pt, off, pos)
            emit_pv(*prev)
            # evict accumulators to SBUF to free the PSUM banks
            evicted = []
            for h in range(H):
                at64 = at_pool.tile([HEAD_DIM, QB], F32, tag=f"at64{h}",
                                    name=f"at64{h}")
                nc.vector.tensor_copy(at64, attn_ps[h][0:HEAD_DIM, :])
                r0 = rec_pool.tile([1, QB], F32, tag=f"r0{h}", name=f"r0{h}")
                nc.vector.tensor_copy(r0, attn_ps[h][HEAD_DIM:HEAD_DIM + 1, :])
                evicted.append((at64, r0))
            return evicted

        def norm_outproj(b_i, j, evicted):
            """Deferred normalization + out-projection for q-block j."""
            at_bj = at_pool.tile([DCORE, QB], BF16, name="at_bj")
            for h, (at64, r0) in enumerate(evicted):
                rf = rec_pool.tile([1, QB], F32, tag=f"rf{h}", name=f"rf{h}")
                nc.vector.reciprocal_approx_fast(rf, r0)
                bc_sb = bc_pool.tile([HEAD_DIM, QB], F32, tag=f"bc{h}",
                                     name=f"bc{h}")
                nc.gpsimd.partition_broadcast(bc_sb, rf)
                nc.vector.tensor_mul(
                    at_bj[HEAD_DIM * h:HEAD_DIM * (h + 1), :], at64, bc_sb)
            for qt in range(QB // 128):
                at = at_bj[:, qt * 128:(qt + 1) * 128]
                ysb = ysb_pool.tile([128, N_EMBED], BF16, tag="ysb",
                                    name="ysb")
                for u in range(N_EMBED // QB):
                    yp = pv_pool.tile([128, QB], F32, tag="aux", name="yp")
                    nc.tensor.matmul(yp, lhsT=at,
                                     rhs=wout_sb[:, u * QB:(u + 1) * QB],
                                     start=True, stop=True)
                    if u == 0:
                        nc.vector.tensor_copy(ysb[:, 0:QB], yp)
                    else:
                        nc.scalar.copy(ysb[:, QB:2 * QB], yp)
                row0 = b_i * seq + j * QB + qt * 128
                nc.sync.dma_start(out=y.ap()[row0:row0 + 128, :], in_=ysb)

        # ---- interleaved schedule: proj frontloaded 3 blocks ahead,
        # norm/out-proj one block behind ----
        n_blocks = B * n_qb
        next_proj = 0
        for _ in range(3):
            if next_proj < n_blocks:
                proj_block(next_proj)
                next_proj += 1
        pending = None
        for b_i in range(B):
            for j in range(n_qb):
                if next_proj < n_blocks:
                    proj_block(next_proj)
                    next_proj += 1
                evicted = attn_kloop(b_i, j)
                if pending is not None:
                    norm_outproj(*pending)
                pending = (b_i, j, evicted)
        norm_outproj(*pending)

    nc.compile()
    return nc


_CACHE = {}


def _get_program(seq=S):
    if seq not in _CACHE:
        _CACHE[seq] = build_program(seq)
    return _CACHE[seq]


def make_in_maps(x, W_qkv, b_qkv, seq=S):
    bf16 = ml_dtypes.bfloat16
    s_tot = B * seq
    xT = np.ascontiguousarray(
        x.reshape(s_tot, N_EMBED).T).astype(bf16)
    in_maps = []
    for c in range(N_CORES):
        csl = slice(DCORE * c, DCORE * (c + 1))
        in_maps.append({
            "xT": xT,
            "wq": np.ascontiguousarray(W_qkv[:, csl]).astype(bf16),
            "wk": np.ascontiguousarray(W_qkv[:, N_EMBED:][:, csl]).astype(bf16),
            "wv": np.ascontiguousarray(W_qkv[:, 2 * N_EMBED:][:, csl]).astype(bf16),
            "bq": np.ascontiguousarray(
                b_qkv[csl].reshape(DCORE, 1)).astype(np.float32),
            "bk": np.ascontiguousarray(
                b_qkv[N_EMBED:][csl].reshape(DCORE, 1)).astype(np.float32),
            "wout": None,  # filled by caller
        })
    return in_maps


def kernel(x, W_qkv, b_qkv, W_out, b_out):
    x = np.asarray(x, dtype=np.float32)
    W_qkv = np.asarray(W_qkv, dtype=np.float32)
    b_qkv = np.asarray(b_qkv, dtype=np.float32)
    W_out = np.asarray(W_out, dtype=np.float32)
    b_out = np.asarray(b_out, dtype=np.float32)

    nc = _get_program(S)
    in_maps = make_in_maps(x, W_qkv, b_qkv, S)
    bf16 = ml_dtypes.bfloat16
    for c in range(N_CORES):
        csl = slice(DCORE * c, DCORE * (c + 1))
        in_maps[c]["wout"] = np.ascontiguousarray(W_out[csl, :]).astype(bf16)

    res = run_bass_kernel_spmd(nc, in_maps, core_ids=list(range(N_CORES)))
    y = np.zeros((B * S, N_EMBED), dtype=np.float32)
    for r in res.results:
        y += r["y"].astype(np.float32)
    # bias + v-bias folded through W_out (softmax rows sum to 1)
    y += b_out[None, :] + b_qkv[2 * N_EMBED:] @ W_out
    return y.reshape(B, S, N_EMBED)
